# revision 8
# baseline (speedup 1.0000x reference)
"""Trainium2 Bass kernel for nn_CI3addFrom01 (segment_reduce).

Reference computation:
    out[b] = sum_m softmax(preweight)[m] * max_k min_j x[b, idx[m,k,j]]
with M = 40704 antichains over DIM = 32, enumerated systematically
(singletons; pairs x {((i,j)), ((i,),(j,))}; triples x 8 types).

Device formulation (no gathers — everything is matmul one-hot gathers,
rectangular broadcast access patterns, and fused multiply-reduce):
  Per core (M-axis sharded 8 ways), per 128-row batch tile:
    G_j = xT.T @ OH_j (j=0,1,2) one-hot matmuls over 838 "set" columns:
        [32 singletons | 62 pairs | 620 triples | 4x31 duplicated pairs X]
    SM = min(G0,G1,G2)   (min over each index set)
    MX = max(G0,G1,G2)   (max over each set; pairs+triples range only)
    xc regions, column-aligned with a host-packed weight row:
      R1  = SM[:, 0:714]                  1-group antichains (+singles core 0)
      R2  = MX[:, 0:682]                  ((i,),(j,)) and ((i,),(j,),(k,))
      R3a = max(SM_sing[a] , SM_pair[p])  grid [32 x 62]   singleton-vs-pair
      R3b = max(X[m,u], X[m,v])           grid [4 x 31 x 31] pair-vs-pair
    partial[b] += sum_c exp(pw_packed[c]) * xc[b, c]   (scalar_tensor_tensor)
  Invalid grid cells get pw = -1e30 (weight 0); R3b counts each antichain
  twice so its pw entries carry -ln(2). Host sums the 8 per-core partials
  and divides by Z = sum of per-core sum(exp(pw_packed)).
"""

import itertools
import math

import numpy as np

DIM = 32
B = 512
NCORES = 8
NPAIR_C = 62
NTRIP_C = 620
NM_C = 4
NTAB = 32 + NPAIR_C + NTRIP_C + NM_C * 31  # 838
NEG = -1e30

PAIRS = list(itertools.combinations(range(DIM), 2))
TRIPS = list(itertools.combinations(range(DIM), 3))
TRIPIDX = {t: i for i, t in enumerate(TRIPS)}

SEG = {}
_off = 0
for _name, _ln in [
    ("R1a", 32), ("R1b", NPAIR_C), ("R1c", NTRIP_C),
    ("R2b", NPAIR_C), ("R2c", NTRIP_C),
    ("R3a", 32 * NPAIR_C), ("R3b", NM_C * 31 * 31),
]:
    SEG[_name] = (_off, _off + _ln)
    _off += _ln
LPACK = _off  # 7224
N_R1 = 32 + NPAIR_C + NTRIP_C  # 714
N_R2 = NPAIR_C + NTRIP_C       # 682
N_R3A = 32 * NPAIR_C           # 1984
N_R3B = NM_C * 31 * 31         # 3844


def _others(m):
    return [x for x in range(DIM) if x != m]


def _table_sets(core):
    sets = [(i, i, i) for i in range(DIM)]
    for p in range(NPAIR_C * core, NPAIR_C * (core + 1)):
        i, j = PAIRS[p]
        sets.append((i, j, j))
    for q in range(NTRIP_C * core, NTRIP_C * (core + 1)):
        sets.append(TRIPS[q])
    for m in range(NM_C * core, NM_C * (core + 1)):
        for x in _others(m):
            sets.append((min(m, x), max(m, x), max(m, x)))
    return sets


_HOST_CACHE = {}


def _onehots(core):
    if ("oh", core) in _HOST_CACHE:
        return _HOST_CACHE[("oh", core)]
    sets = np.asarray(_table_sets(core), dtype=np.int64)
    oh = np.zeros((3, DIM, NTAB), dtype=np.float32)
    for j in range(3):
        oh[j, sets[:, j], np.arange(NTAB)] = 1.0
    _HOST_CACHE[("oh", core)] = oh
    return oh


def _widx_r3a(core):
    if ("r3a", core) in _HOST_CACHE:
        return _HOST_CACHE[("r3a", core)]
    g = np.full((32, NPAIR_C), -1, dtype=np.int64)
    for a in range(32):
        for pl, p in enumerate(range(NPAIR_C * core, NPAIR_C * (core + 1))):
            b, c = PAIRS[p]
            if a == b or a == c:
                continue
            tri = tuple(sorted((a, b, c)))
            g[a, pl] = 1024 + 8 * TRIPIDX[tri] + 2 + tri.index(a)
    _HOST_CACHE[("r3a", core)] = g
    return g


def _widx_r3b(core):
    if ("r3b", core) in _HOST_CACHE:
        return _HOST_CACHE[("r3b", core)]
    g = np.full((NM_C, 31, 31), -1, dtype=np.int64)
    for ml, m in enumerate(range(NM_C * core, NM_C * (core + 1))):
        ot = _others(m)
        for u in range(31):
            for v in range(31):
                if u == v:
                    continue
                tri = tuple(sorted((m, ot[u], ot[v])))
                g[ml, u, v] = 1024 + 8 * TRIPIDX[tri] + (7, 5, 6)[tri.index(m)]
    _HOST_CACHE[("r3b", core)] = g
    return g


def _packed_pw(core, pw):
    pw = np.asarray(pw, dtype=np.float64).reshape(-1)
    out = np.full(LPACK, NEG, dtype=np.float64)
    if core == 0:
        out[SEG["R1a"][0]:SEG["R1a"][1]] = pw[0:32]
    p0, p1 = 32 + 2 * NPAIR_C * core, 32 + 2 * NPAIR_C * (core + 1)
    out[SEG["R1b"][0]:SEG["R1b"][1]] = pw[p0:p1:2]
    out[SEG["R2b"][0]:SEG["R2b"][1]] = pw[p0 + 1:p1 + 1:2]
    t0, t1 = 1024 + 8 * NTRIP_C * core, 1024 + 8 * NTRIP_C * (core + 1)
    out[SEG["R1c"][0]:SEG["R1c"][1]] = pw[t0:t1:8]
    out[SEG["R2c"][0]:SEG["R2c"][1]] = pw[t0 + 1:t1 + 1:8]
    for name, grid, dup in [("R3a", _widx_r3a(core), False),
                            ("R3b", _widx_r3b(core), True)]:
        flat = grid.reshape(-1)
        vals = np.full(flat.shape, NEG, dtype=np.float64)
        ok = flat >= 0
        vals[ok] = pw[flat[ok]] - (math.log(2.0) if dup else 0.0)
        s, e = SEG[name]
        out[s:e] = vals
    return out.astype(np.float32)


def _expected_idx():
    acs = [((i,),) for i in range(DIM)]
    for i, j in PAIRS:
        acs.append(((i, j),))
        acs.append(((i,), (j,)))
    for i, j, k in TRIPS:
        acs += [((i, j, k),), ((i,), (j,), (k,)), ((i,), (j, k)), ((j,), (i, k)),
                ((k,), (i, j)), ((i, j), (j, k)), ((i, k), (j, k)), ((i, j), (i, k))]
    idx = np.zeros((len(acs), 3, 3), dtype=np.int32)
    for m, ac in enumerate(acs):
        groups = [list(g) + [g[-1]] * (3 - len(g)) for g in ac]
        while len(groups) < 3:
            groups.append(groups[-1])
        idx[m] = np.array(groups, dtype=np.int32)
    return idx


_NC_CACHE = {}


def _build_nc(reps=1):
    import concourse.mybir as mybir
    from concourse import bacc
    from concourse.tile import TileContext

    f32 = mybir.dt.float32
    bf16 = mybir.dt.bfloat16
    Alu = mybir.AluOpType

    nc = bacc.Bacc(None, target_bir_lowering=False, debug=False)
    xT_d = nc.dram_tensor("xT", [DIM, B], f32, kind="ExternalInput")
    oh_d = nc.dram_tensor("oh", [DIM, 3 * NTAB], f32, kind="ExternalInput")
    pwp_d = nc.dram_tensor("pwp", [1, LPACK], f32, kind="ExternalInput")
    out_d = nc.dram_tensor("out", [B, 1], f32, kind="ExternalOutput")
    zsum_d = nc.dram_tensor("zsum", [128, 1], f32, kind="ExternalOutput")

    with TileContext(nc) as tc:
        with (
            tc.tile_pool(name="const", bufs=1) as cp,
            tc.tile_pool(name="work", bufs=2) as wp,
            tc.tile_pool(name="junkp", bufs=1) as jp,
            tc.tile_pool(name="pe", bufs=1, space="PSUM") as pe_pool,
            tc.tile_pool(name="pg", bufs=1, space="PSUM") as pg_pool,
        ):
            oh_t = cp.tile([DIM, 3 * NTAB], f32)
            xt_t = cp.tile([DIM, B], f32)
            pwp_t = cp.tile([1, LPACK], f32)
            ones1 = cp.tile([1, 128], f32)
            E = cp.tile([128, LPACK], bf16)
            zparts = cp.tile([128, (LPACK + 511) // 512], f32)
            outb = cp.tile([128, 4], f32)
            nc.sync.dma_start(oh_t[:], oh_d[:])
            nc.sync.dma_start(xt_t[:], xT_d[:])
            nc.sync.dma_start(pwp_t[:], pwp_d[:])
            nc.vector.memset(ones1[:], 1.0)

            rep_blocks(nc, tc, mybir, f32, bf16, Alu, cp, wp, jp, pe_pool,
                       pg_pool, oh_t, xt_t, pwp_t, ones1, E, zparts, outb,
                       reps)

            # per-core softmax denominator: sum of the per-chunk exp accums
            zacc = cp.tile([128, 1], f32)
            nc.vector.tensor_reduce(zacc[:], zparts[:],
                                    axis=mybir.AxisListType.X, op=Alu.add)
            nc.sync.dma_start(zsum_d[:], zacc[:])

            for t in range(4):
                nc.sync.dma_start(out_d[t * 128:(t + 1) * 128, :],
                                  outb[:, t:t + 1])
    nc.finalize()
    return nc


def rep_blocks(nc, tc, mybir, f32, bf16, Alu, cp, wp, jp, pe_pool, pg_pool,
               oh_t, xt_t, pwp_t, ones1, E, zparts, outb, reps):
    for _rep in range(reps):
            # E = exp(pw_packed) broadcast to 128 partitions via ones-matmul;
            # per-chunk accum feeds the softmax denominator for free
            for ci, s in enumerate(range(0, LPACK, 512)):
                e = min(s + 512, LPACK)
                eb = pe_pool.tile([128, 512], f32, tag="eb")
                nc.tensor.matmul(eb[:, : e - s], ones1[:], pwp_t[:, s:e],
                                 start=True, stop=True)
                nc.scalar.activation(E[:, s:e], eb[:, : e - s],
                                     mybir.ActivationFunctionType.Exp,
                                     accum_out=zparts[:, ci:ci + 1])

            for t in range(4):
                g = []
                for j in range(3):
                    gj = pg_pool.tile([128, NTAB], f32, tag=f"g{j}")
                    for s in range(0, NTAB, 512):
                        e = min(s + 512, NTAB)
                        nc.tensor.matmul(
                            gj[:, s:e], xt_t[:, t * 128:(t + 1) * 128],
                            oh_t[:, j * NTAB + s: j * NTAB + e],
                            start=True, stop=True)
                    g.append(gj)

                # tensor_tensor may read at most one PSUM operand: copy G0 out
                # (on ACT — otherwise idle, keeps DVE free)
                c0 = wp.tile([128, NTAB], bf16, tag="c0")
                nc.scalar.copy(c0[:], g[0][:])
                t1 = wp.tile([128, NTAB], bf16, tag="t1")
                sm = wp.tile([128, NTAB], bf16, tag="sm")
                nc.vector.tensor_tensor(t1[:], c0[:], g[1][:], Alu.min)
                nc.vector.tensor_tensor(sm[:], t1[:], g[2][:], Alu.min)
                t2 = wp.tile([128, N_R2], bf16, tag="t2")
                mx = wp.tile([128, N_R2], bf16, tag="mx")
                nc.vector.tensor_tensor(t2[:], c0[:, 32:N_R1],
                                        g[1][:, 32:N_R1], Alu.max)
                nc.vector.tensor_tensor(mx[:], t2[:], g[2][:, 32:N_R1], Alu.max)

                r3a = wp.tile([128, 32, NPAIR_C], bf16, tag="r3a")
                nc.vector.tensor_tensor(
                    r3a[:],
                    sm[:, 0:32].unsqueeze(2).broadcast_to([128, 32, NPAIR_C]),
                    sm[:, 32:32 + NPAIR_C].unsqueeze(1)
                    .broadcast_to([128, 32, NPAIR_C]),
                    Alu.max)
                r3b = wp.tile([128, NM_C, 31, 31], bf16, tag="r3b")
                xv = sm[:, N_R1:NTAB].rearrange("p (m t) -> p m t", m=NM_C)
                nc.vector.tensor_tensor(
                    r3b[:],
                    xv.unsqueeze(3).broadcast_to([128, NM_C, 31, 31]),
                    xv.unsqueeze(2).broadcast_to([128, NM_C, 31, 31]),
                    Alu.max)

                junk = jp.tile([128, N_R3B], bf16, tag="junk")
                acc = wp.tile([128, 4], f32, tag="acc")
                for i, (xc, seg) in enumerate([
                    (sm[:, 0:N_R1], (0, N_R1)),
                    (mx[:], (N_R1, N_R1 + N_R2)),
                    (r3a[:].rearrange("p a q -> p (a q)"), SEG["R3a"]),
                    (r3b[:].rearrange("p m u v -> p (m u v)"), SEG["R3b"]),
                ]):
                    s, e = seg
                    nc.vector.scalar_tensor_tensor(
                        junk[:, : e - s], xc, 1.0, E[:, s:e],
                        op0=Alu.mult, op1=Alu.mult, accum_out=acc[:, i:i + 1])
                nc.vector.tensor_reduce(outb[:, t:t + 1], acc[:],
                                        axis=mybir.AxisListType.X, op=Alu.add)


def kernel(x, preweight, idx):
    from concourse.bass_utils import run_bass_kernel_spmd

    x = np.ascontiguousarray(np.asarray(x, dtype=np.float32))
    pw = np.asarray(preweight, dtype=np.float32).reshape(-1)
    idx = np.asarray(idx)
    if not np.array_equal(idx, _expected_idx()):
        raise ValueError("idx does not match the expected antichain table")

    if "nc" not in _NC_CACHE:
        _NC_CACHE["nc"] = _build_nc()
    nc = _NC_CACHE["nc"]

    xT = np.ascontiguousarray(x.T)  # [32, 512]
    in_maps = []
    for core in range(NCORES):
        oh = _onehots(core)  # [3, 32, NTAB]
        in_maps.append({
            "xT": xT,
            "oh": np.ascontiguousarray(
                oh.transpose(1, 0, 2).reshape(DIM, 3 * NTAB)),
            "pwp": _packed_pw(core, pw).reshape(1, LPACK),
        })

    res = run_bass_kernel_spmd(nc, in_maps, core_ids=list(range(NCORES)))
    total = np.zeros((B, 1), dtype=np.float64)
    z = 0.0
    for r in res.results:
        total += r["out"].astype(np.float64)
        z += float(r["zsum"][0, 0])
    return (total / z).astype(np.float32)


if __name__ == "__main__":
    rng = np.random.default_rng(11)
    x = rng.standard_normal((B, DIM)).astype(np.float32)
    pw = rng.standard_normal((1, 40704)).astype(np.float32)
    out = kernel(x, pw, _expected_idx())
    print("out", out.shape, out[:4, 0])


# revision 9
# speedup vs baseline: 88.9304x; 88.9304x over previous
"""Trainium2 Bass kernel for nn_CI3addFrom01 (segment_reduce).

Reference computation:
    out[b] = sum_m softmax(preweight)[m] * max_k min_j x[b, idx[m,k,j]]
with M = 40704 antichains over DIM = 32, enumerated systematically
(singletons; pairs x {((i,j)), ((i,),(j,))}; triples x 8 types).

Device formulation (no gathers — everything is matmul one-hot gathers,
rectangular broadcast access patterns, and fused multiply-reduce):
  Per core (M-axis sharded 8 ways), per 128-row batch tile:
    G_j = xT.T @ OH_j (j=0,1,2) one-hot matmuls over 838 "set" columns:
        [32 singletons | 62 pairs | 620 triples | 4x31 duplicated pairs X]
    SM = min(G0,G1,G2)   (min over each index set)
    MX = max(G0,G1,G2)   (max over each set; pairs+triples range only)
    xc regions, column-aligned with a host-packed weight row:
      R1  = SM[:, 0:714]                  1-group antichains (+singles core 0)
      R2  = MX[:, 0:682]                  ((i,),(j,)) and ((i,),(j,),(k,))
      R3a = max(SM_sing[a] , SM_pair[p])  grid [32 x 62]   singleton-vs-pair
      R3b = max(X[m,u], X[m,v])           grid [4 x 31 x 31] pair-vs-pair
    partial[b] += sum_c exp(pw_packed[c]) * xc[b, c]   (scalar_tensor_tensor)
  Invalid grid cells get pw = -1e30 (weight 0); R3b counts each antichain
  twice so its pw entries carry -ln(2). Host sums the 8 per-core partials
  and divides by Z = sum of per-core sum(exp(pw_packed)).
"""

import itertools
import math

import numpy as np

DIM = 32
B = 512
NCORES = 8
NPAIR_C = 62
NTRIP_C = 620
NM_C = 4
NTAB = 32 + NPAIR_C + NTRIP_C + NM_C * 31  # 838
NEG = -1e30

PAIRS = list(itertools.combinations(range(DIM), 2))
TRIPS = list(itertools.combinations(range(DIM), 3))
TRIPIDX = {t: i for i, t in enumerate(TRIPS)}

SEG = {}
_off = 0
for _name, _ln in [
    ("R1a", 32), ("R1b", NPAIR_C), ("R1c", NTRIP_C),
    ("R2b", NPAIR_C), ("R2c", NTRIP_C),
    ("R3a", 32 * NPAIR_C), ("R3b", NM_C * 31 * 31),
]:
    SEG[_name] = (_off, _off + _ln)
    _off += _ln
LPACK = _off  # 7224
N_R1 = 32 + NPAIR_C + NTRIP_C  # 714
N_R2 = NPAIR_C + NTRIP_C       # 682
N_R3A = 32 * NPAIR_C           # 1984
N_R3B = NM_C * 31 * 31         # 3844


def _others(m):
    return [x for x in range(DIM) if x != m]


def _table_sets(core):
    sets = [(i, i, i) for i in range(DIM)]
    for p in range(NPAIR_C * core, NPAIR_C * (core + 1)):
        i, j = PAIRS[p]
        sets.append((i, j, j))
    for q in range(NTRIP_C * core, NTRIP_C * (core + 1)):
        sets.append(TRIPS[q])
    for m in range(NM_C * core, NM_C * (core + 1)):
        for x in _others(m):
            sets.append((min(m, x), max(m, x), max(m, x)))
    return sets


_HOST_CACHE = {}


def _onehots(core):
    if ("oh", core) in _HOST_CACHE:
        return _HOST_CACHE[("oh", core)]
    sets = np.asarray(_table_sets(core), dtype=np.int64)
    oh = np.zeros((3, DIM, NTAB), dtype=np.float32)
    for j in range(3):
        oh[j, sets[:, j], np.arange(NTAB)] = 1.0
    _HOST_CACHE[("oh", core)] = oh
    return oh


def _widx_r3a(core):
    if ("r3a", core) in _HOST_CACHE:
        return _HOST_CACHE[("r3a", core)]
    g = np.full((32, NPAIR_C), -1, dtype=np.int64)
    for a in range(32):
        for pl, p in enumerate(range(NPAIR_C * core, NPAIR_C * (core + 1))):
            b, c = PAIRS[p]
            if a == b or a == c:
                continue
            tri = tuple(sorted((a, b, c)))
            g[a, pl] = 1024 + 8 * TRIPIDX[tri] + 2 + tri.index(a)
    _HOST_CACHE[("r3a", core)] = g
    return g


def _widx_r3b(core):
    if ("r3b", core) in _HOST_CACHE:
        return _HOST_CACHE[("r3b", core)]
    g = np.full((NM_C, 31, 31), -1, dtype=np.int64)
    for ml, m in enumerate(range(NM_C * core, NM_C * (core + 1))):
        ot = _others(m)
        for u in range(31):
            for v in range(31):
                if u == v:
                    continue
                tri = tuple(sorted((m, ot[u], ot[v])))
                g[ml, u, v] = 1024 + 8 * TRIPIDX[tri] + (7, 5, 6)[tri.index(m)]
    _HOST_CACHE[("r3b", core)] = g
    return g


def _packed_pw(core, pw):
    pw = np.asarray(pw, dtype=np.float64).reshape(-1)
    out = np.full(LPACK, NEG, dtype=np.float64)
    if core == 0:
        out[SEG["R1a"][0]:SEG["R1a"][1]] = pw[0:32]
    p0, p1 = 32 + 2 * NPAIR_C * core, 32 + 2 * NPAIR_C * (core + 1)
    out[SEG["R1b"][0]:SEG["R1b"][1]] = pw[p0:p1:2]
    out[SEG["R2b"][0]:SEG["R2b"][1]] = pw[p0 + 1:p1 + 1:2]
    t0, t1 = 1024 + 8 * NTRIP_C * core, 1024 + 8 * NTRIP_C * (core + 1)
    out[SEG["R1c"][0]:SEG["R1c"][1]] = pw[t0:t1:8]
    out[SEG["R2c"][0]:SEG["R2c"][1]] = pw[t0 + 1:t1 + 1:8]
    for name, grid, dup in [("R3a", _widx_r3a(core), False),
                            ("R3b", _widx_r3b(core), True)]:
        flat = grid.reshape(-1)
        vals = np.full(flat.shape, NEG, dtype=np.float64)
        ok = flat >= 0
        vals[ok] = pw[flat[ok]] - (math.log(2.0) if dup else 0.0)
        s, e = SEG[name]
        out[s:e] = vals
    return out.astype(np.float32)


def _expected_idx():
    acs = [((i,),) for i in range(DIM)]
    for i, j in PAIRS:
        acs.append(((i, j),))
        acs.append(((i,), (j,)))
    for i, j, k in TRIPS:
        acs += [((i, j, k),), ((i,), (j,), (k,)), ((i,), (j, k)), ((j,), (i, k)),
                ((k,), (i, j)), ((i, j), (j, k)), ((i, k), (j, k)), ((i, j), (i, k))]
    idx = np.zeros((len(acs), 3, 3), dtype=np.int32)
    for m, ac in enumerate(acs):
        groups = [list(g) + [g[-1]] * (3 - len(g)) for g in ac]
        while len(groups) < 3:
            groups.append(groups[-1])
        idx[m] = np.array(groups, dtype=np.int32)
    return idx


_NC_CACHE = {}


def _build_nc(reps=1):
    import concourse.mybir as mybir
    from concourse import bacc
    from concourse.tile import TileContext

    f32 = mybir.dt.float32
    bf16 = mybir.dt.bfloat16
    Alu = mybir.AluOpType

    nc = bacc.Bacc(None, target_bir_lowering=False, debug=False)
    xT_d = nc.dram_tensor("xT", [DIM, B], bf16, kind="ExternalInput")
    oh_d = nc.dram_tensor("oh", [DIM, 3 * NTAB], bf16, kind="ExternalInput")
    pwp_d = nc.dram_tensor("pwp", [1, LPACK], bf16, kind="ExternalInput")
    out_d = nc.dram_tensor("out", [B, 1], f32, kind="ExternalOutput")
    zsum_d = nc.dram_tensor("zsum", [128, 1], f32, kind="ExternalOutput")

    with TileContext(nc) as tc:
        with (
            tc.tile_pool(name="const", bufs=1) as cp,
            tc.tile_pool(name="work", bufs=2) as wp,
            tc.tile_pool(name="junkp", bufs=1) as jp,
            tc.tile_pool(name="pe", bufs=1, space="PSUM") as pe_pool,
            tc.tile_pool(name="pg", bufs=1, space="PSUM") as pg_pool,
        ):
            oh_t = cp.tile([DIM, 3 * NTAB], bf16)
            xt_t = cp.tile([DIM, B], bf16)
            pwp_t = cp.tile([1, LPACK], bf16)
            ones1 = cp.tile([1, 128], bf16)
            E = cp.tile([128, LPACK], bf16)
            zparts = cp.tile([128, (LPACK + 511) // 512], f32)
            outb = cp.tile([128, 4], f32)
            nc.sync.dma_start(oh_t[:], oh_d[:])
            nc.sync.dma_start(xt_t[:], xT_d[:])
            nc.sync.dma_start(pwp_t[:], pwp_d[:])
            nc.vector.memset(ones1[:], 1.0)

            rep_blocks(nc, tc, mybir, f32, bf16, Alu, cp, wp, jp, pe_pool,
                       pg_pool, oh_t, xt_t, pwp_t, ones1, E, zparts, outb,
                       reps)

            # per-core softmax denominator: sum of the per-chunk exp accums
            zacc = cp.tile([128, 1], f32)
            nc.vector.tensor_reduce(zacc[:], zparts[:],
                                    axis=mybir.AxisListType.X, op=Alu.add)
            nc.sync.dma_start(zsum_d[:], zacc[:])

            for t in range(4):
                nc.sync.dma_start(out_d[t * 128:(t + 1) * 128, :],
                                  outb[:, t:t + 1])
    nc.finalize()
    return nc


def rep_blocks(nc, tc, mybir, f32, bf16, Alu, cp, wp, jp, pe_pool, pg_pool,
               oh_t, xt_t, pwp_t, ones1, E, zparts, outb, reps):
    for _rep in range(reps):
            # E = exp(pw_packed) broadcast to 128 partitions via ones-matmul;
            # per-chunk accum feeds the softmax denominator for free
            for ci, s in enumerate(range(0, LPACK, 512)):
                e = min(s + 512, LPACK)
                eb = pe_pool.tile([128, 512], f32, tag="eb")
                nc.tensor.matmul(eb[:, : e - s], ones1[:], pwp_t[:, s:e],
                                 start=True, stop=True)
                nc.scalar.activation(E[:, s:e], eb[:, : e - s],
                                     mybir.ActivationFunctionType.Exp,
                                     accum_out=zparts[:, ci:ci + 1])

            for t in range(4):
                g = []
                for j in range(3):
                    gj = pg_pool.tile([128, NTAB], f32, tag=f"g{j}")
                    for s in range(0, NTAB, 512):
                        e = min(s + 512, NTAB)
                        nc.tensor.matmul(
                            gj[:, s:e], xt_t[:, t * 128:(t + 1) * 128],
                            oh_t[:, j * NTAB + s: j * NTAB + e],
                            start=True, stop=True)
                    g.append(gj)

                # tensor_tensor may read at most one PSUM operand: copy G0 out
                # (on ACT — otherwise idle, keeps DVE free)
                c0 = wp.tile([128, NTAB], bf16, tag="c0")
                nc.scalar.copy(c0[:], g[0][:])
                t1 = wp.tile([128, NTAB], bf16, tag="t1")
                sm = wp.tile([128, NTAB], bf16, tag="sm")
                nc.vector.tensor_tensor(t1[:], c0[:], g[1][:], Alu.min)
                nc.vector.tensor_tensor(sm[:], t1[:], g[2][:], Alu.min)
                t2 = wp.tile([128, N_R2], bf16, tag="t2")
                mx = wp.tile([128, N_R2], bf16, tag="mx")
                nc.vector.tensor_tensor(t2[:], c0[:, 32:N_R1],
                                        g[1][:, 32:N_R1], Alu.max)
                nc.vector.tensor_tensor(mx[:], t2[:], g[2][:, 32:N_R1], Alu.max)

                r3a = wp.tile([128, 32, NPAIR_C], bf16, tag="r3a")
                nc.vector.tensor_tensor(
                    r3a[:],
                    sm[:, 0:32].unsqueeze(2).broadcast_to([128, 32, NPAIR_C]),
                    sm[:, 32:32 + NPAIR_C].unsqueeze(1)
                    .broadcast_to([128, 32, NPAIR_C]),
                    Alu.max)
                r3b = wp.tile([128, NM_C, 31, 31], bf16, tag="r3b")
                xv = sm[:, N_R1:NTAB].rearrange("p (m t) -> p m t", m=NM_C)
                nc.vector.tensor_tensor(
                    r3b[:],
                    xv.unsqueeze(3).broadcast_to([128, NM_C, 31, 31]),
                    xv.unsqueeze(2).broadcast_to([128, NM_C, 31, 31]),
                    Alu.max)

                junk = jp.tile([128, N_R3B], bf16, tag="junk")
                acc = wp.tile([128, 4], f32, tag="acc")
                for i, (xc, seg) in enumerate([
                    (sm[:, 0:N_R1], (0, N_R1)),
                    (mx[:], (N_R1, N_R1 + N_R2)),
                    (r3a[:].rearrange("p a q -> p (a q)"), SEG["R3a"]),
                    (r3b[:].rearrange("p m u v -> p (m u v)"), SEG["R3b"]),
                ]):
                    s, e = seg
                    nc.vector.scalar_tensor_tensor(
                        junk[:, : e - s], xc, 1.0, E[:, s:e],
                        op0=Alu.mult, op1=Alu.mult, accum_out=acc[:, i:i + 1])
                nc.vector.tensor_reduce(outb[:, t:t + 1], acc[:],
                                        axis=mybir.AxisListType.X, op=Alu.add)


def make_in_maps(x, pw):
    import ml_dtypes

    bf = ml_dtypes.bfloat16
    xT = np.ascontiguousarray(np.asarray(x, np.float32).T.astype(bf))
    in_maps = []
    for core in range(NCORES):
        oh = _onehots(core)  # [3, 32, NTAB]
        in_maps.append({
            "xT": xT,
            "oh": np.ascontiguousarray(
                oh.transpose(1, 0, 2).reshape(DIM, 3 * NTAB).astype(bf)),
            "pwp": _packed_pw(core, pw).reshape(1, LPACK).astype(bf),
        })
    return in_maps


def kernel(x, preweight, idx):
    from concourse.bass_utils import run_bass_kernel_spmd

    x = np.ascontiguousarray(np.asarray(x, dtype=np.float32))
    pw = np.asarray(preweight, dtype=np.float32).reshape(-1)
    idx = np.asarray(idx)
    if not np.array_equal(idx, _expected_idx()):
        raise ValueError("idx does not match the expected antichain table")

    if "nc" not in _NC_CACHE:
        _NC_CACHE["nc"] = _build_nc()
    nc = _NC_CACHE["nc"]

    in_maps = make_in_maps(x, pw)
    res = run_bass_kernel_spmd(nc, in_maps, core_ids=list(range(NCORES)))
    total = np.zeros((B, 1), dtype=np.float64)
    z = 0.0
    for r in res.results:
        total += r["out"].astype(np.float64)
        z += float(r["zsum"][0, 0])
    return (total / z).astype(np.float32)


if __name__ == "__main__":
    rng = np.random.default_rng(11)
    x = rng.standard_normal((B, DIM)).astype(np.float32)
    pw = rng.standard_normal((1, 40704)).astype(np.float32)
    out = kernel(x, pw, _expected_idx())
    print("out", out.shape, out[:4, 0])
